# revision 1
# baseline (speedup 1.0000x reference)
"""Trainium2 Bass kernel for nn_AdvancedGCN (GCN -> GAT -> EdgeConv -> GIN ->
global-attention pooling) over N=50000 nodes / E=800000 edges, SPMD on 8
NeuronCores.

Strategy: nodes are sharded 6250/core (padded to 6272 = 49 blocks of 128) and
sorted by in-degree so each 128-node block pads its in-edge list to the block
max degree.  All graph index work happens on host in numpy and is baked into
int32 gather-index tables; the device program is pure dense compute:

 - per block ONE indirect DMA gathers all K*128 neighbor rows from a
   replicated DRAM table (bf16 rows, 256/272 B; accumulation stays f32),
 - segment reductions are in-place strided tree folds on the vector engine,
 - GCN's edge norm dinv[src]*dinv[dst] is separable: dinv[src] is folded into
   the z-table write, dinv[dst] is a per-partition scalar,
 - GAT edge softmax runs in one pass without max subtraction (values are
   tiny); sentinel rows with a_src=-1e30 make padded slots contribute exp->0,
 - EdgeConv is rewritten msg = relu(u[src] + v[dst]) @ W2 with u = h2 @ W1b,
   v = h2 @ (W1a-W1b) + b1 precomputed per node; padded slots duplicate a
   real edge so segment-max is exact,
 - node tables are replicated with AllGather; pooling is a [64,129]
   matmul-accumulated partial reduced with one AllReduce.
"""
import os
import sys

import numpy as np

for _p in ("/opt/trn_rl_repo", "/root/.axon_site/_ro/trn_rl_repo"):
    if os.path.isdir(_p) and _p not in sys.path:
        sys.path.insert(0, _p)

try:  # persistent XLA executable cache: identical programs skip neuronxcc
    import jax
    jax.config.update("jax_compilation_cache_dir", "/tmp/jaxcache_gnn")
    jax.config.update("jax_persistent_cache_min_entry_size_bytes", -1)
    jax.config.update("jax_persistent_cache_min_compile_time_secs", 0)
except Exception:
    pass

import concourse.bass as bass
import concourse.bacc as bacc
import concourse.tile as tile
import concourse.mybir as mybir
from concourse.bass_utils import run_bass_kernel_spmd
from concourse.masks import make_identity

N, E, IN, H, G, OUT = 50000, 800000, 128, 128, 64, 10
HEADS, C = 4, 32
R = 8                    # cores
NPC = N // R             # 6250 nodes per core
NB = (NPC + 127) // 128  # 49 blocks per core
NPCP = NB * 128          # 6272 padded nodes per core
TABR = R * NPCP          # replicated table rows
GW = 136                 # g-table row width (128 g + 4 a_src + 4 pad)
NEG = -1.0e30
f32, i32 = mybir.dt.float32, mybir.dt.int32
bf16 = mybir.dt.bfloat16
AF = mybir.ActivationFunctionType
OP = mybir.AluOpType
REPL = [list(range(R))]

# AllGather chunking (in blocks) so table AG overlaps the producing stage.
CHUNKS = (13, 12, 12, 12)
assert sum(CHUNKS) == NB
_chunk_start = np.cumsum((0,) + CHUNKS)[:-1]          # first block of chunk
_chunk_rows = np.array(CHUNKS) * 128                  # per-core rows in chunk
_chunk_tab_base = np.cumsum([0] + [R * r for r in _chunk_rows])[:-1]
_chunk_of_block = np.repeat(np.arange(len(CHUNKS)), CHUNKS)


def _row_of_gslot(gs):
    """Global slot id (r*NPCP + s) -> replicated-table row (chunked layout)."""
    gs = np.asarray(gs)
    r, s = gs // NPCP, gs % NPCP
    b = s // 128
    c = _chunk_of_block[b]
    return (_chunk_tab_base[c] + r * _chunk_rows[c]
            + (s - 128 * _chunk_start[c])).astype(np.int32)


def _csr_tables(es, ed, slot_of, row_of_node, sent_row, dup_pad):
    """Build per-core padded-CSR gather tables for edges (es -> ed).

    Returns (K[b] common block slot counts, off[b] col offsets, idx [R,128,S])
    where idx[r,p,off[b]+k] = table row of the k-th in-neighbor of the node in
    core r, block b, partition p.  Padding is the slot-0 edge (dup_pad) or the
    core's sentinel row.
    """
    sg = slot_of[ed]                       # global dst slot
    # secondary key: ascending table row of the source -> each partition's
    # descriptor stream walks HBM mostly forward (better row-buffer locality)
    order = np.lexsort((row_of_node[es], sg))
    es_s, sg_s = es[order], sg[order]
    counts = np.bincount(sg_s, minlength=R * NPCP)
    starts = np.concatenate(([0], np.cumsum(counts)))[:-1]
    k_of = np.arange(len(sg_s)) - starts[sg_s]
    K = counts.reshape(R, NB, 128).max(axis=(0, 2))   # common across cores
    off = np.concatenate(([0], np.cumsum(K)))
    S = int(off[-1])
    idx = np.empty((R, 128, S), np.int32)
    idx[:] = sent_row[:, None, None]
    r_e, s_e = sg_s // NPCP, sg_s % NPCP
    b_e, p_e = s_e // 128, s_e % 128
    idx[r_e, p_e, off[b_e] + k_of] = row_of_node[es_s]
    if dup_pad:
        # replace sentinel padding with a copy of the slot-0 edge (exact for
        # segment-max); slots with zero edges keep the sentinel.
        cnt = counts.reshape(R, NB, 128)
        for b in range(NB):
            kb = int(K[b])
            if kb == 0:
                continue
            cols = np.arange(off[b], off[b] + kb)
            # duplicate the LAST (highest-row) edge: keeps each partition's
            # descriptor stream monotone in HBM; max() result is identical
            lastc = off[b] + np.maximum(cnt[:, b, :] - 1, 0)   # [R,128]
            last = np.take_along_axis(
                idx, lastc[:, :, None], axis=2)                # [R,128,1]
            have = (cnt[:, b, :, None] > np.arange(kb))    # [R,128,kb] valid
            nonzero = cnt[:, b, :, None] > 0
            blk = idx[:, :, cols]
            idx[:, :, cols] = np.where(have, blk, np.where(nonzero, last, blk))
    return K, off, idx


def _preprocess(x, edge_index, batch, gcn_W, gcn_b, gat_W, att_src, att_dst,
                gat_b, ec_W1, ec_b1, ec_W2, ec_b2, gin_W1, gin_b1, gin_W2,
                gin_b2, gate_W1, gate_b1, gate_W2, gate_b2, fc_W, fc_b):
    src = np.asarray(edge_index[0], np.int64)
    dst = np.asarray(edge_index[1], np.int64)
    x = np.asarray(x, np.float32)
    batch = np.asarray(batch, np.int64)

    deg2 = np.bincount(dst, minlength=N)            # in-degree w/o self-loop
    dinv = (1.0 / np.sqrt((deg2 + 1).astype(np.float64))).astype(np.float32)

    # per-core permutation: sort own nodes by in-degree descending
    perm = np.empty((R, NPC), np.int64)
    for r in range(R):
        base = r * NPC
        perm[r] = base + np.argsort(-deg2[base:base + NPC], kind="stable")
    slot_of = np.empty(N, np.int64)                 # node -> global slot
    for r in range(R):
        slot_of[perm[r]] = r * NPCP + np.arange(NPC)
    row_of_node = _row_of_gslot(slot_of)            # node -> table row
    sent_row = _row_of_gslot(np.arange(R) * NPCP + (NPCP - 1))

    loops = np.arange(N)
    es1 = np.concatenate([src, loops])
    ed1 = np.concatenate([dst, loops])
    K1, off1, idx1 = _csr_tables(es1, ed1, slot_of, row_of_node, sent_row,
                                 dup_pad=False)
    K2, off2, idx2ec = _csr_tables(src, dst, slot_of, row_of_node, sent_row,
                                   dup_pad=True)
    _, _, idx2gin = _csr_tables(src, dst, slot_of, row_of_node, sent_row,
                                dup_pad=False)
    if (deg2 == 0).any():
        print("kernel.py WARNING: %d nodes with zero in-degree; EdgeConv "
              "aggregation approximates relu(b2)->0 for them" %
              int((deg2 == 0).sum()))

    # per-core node-aligned params
    xs, dinvs, bidss = [], [], []
    for r in range(R):
        xp = np.zeros((NPCP, IN), np.float32)
        xp[:NPC] = x[perm[r]]
        xs.append(np.ascontiguousarray(xp.T))   # [128, NPCP] transposed
        dv = np.zeros((NB * 128,), np.float32)
        dv[:NPC] = dinv[perm[r]]
        dinvs.append(dv.reshape(NB, 128).T.copy())        # [128, NB]
        bd = np.full((NB * 128,), 999.0, np.float32)
        bd[:NPC] = batch[perm[r]].astype(np.float32)
        bidss.append(bd.reshape(NB, 128).T.copy())        # [128, NB]

    # derived weights (host)
    gat_W = np.asarray(gat_W, np.float32)
    att_src = np.asarray(att_src, np.float32)
    att_dst = np.asarray(att_dst, np.float32)
    B_src = np.einsum("fhc,hc->fh",
                      gat_W.reshape(IN, HEADS, C), att_src).astype(np.float32)
    B_dst = np.einsum("fhc,hc->fh",
                      gat_W.reshape(IN, HEADS, C), att_dst).astype(np.float32)
    ec_W1 = np.asarray(ec_W1, np.float32)
    W1a, W1b = ec_W1[:H], ec_W1[H:]
    W1d = (W1a - W1b).astype(np.float32)

    const = {
        "gcn_W": np.asarray(gcn_W, np.float32),
        "gat_W": gat_W,
        "B_src": B_src, "B_dst": B_dst,
        "W1b": np.ascontiguousarray(W1b), "W1d": W1d,
        "ec_W2": np.asarray(ec_W2, np.float32),
        "gin_W1": np.asarray(gin_W1, np.float32),
        "gin_W2": np.asarray(gin_W2, np.float32),
        "gate_W1": np.asarray(gate_W1, np.float32),
        "gate_W2": np.asarray(gate_W2, np.float32).reshape(H, 1),
        "fc_W": np.asarray(fc_W, np.float32),
        "fc_b": np.asarray(fc_b, np.float32).reshape(1, OUT),
        "gcnb_bc": np.tile(np.asarray(gcn_b, np.float32), (128, 1)),
        "gatb_bc": np.tile(np.asarray(gat_b, np.float32), (128, 1)),
        "ecb1_bc": np.tile(np.asarray(ec_b1, np.float32), (128, 1)),
        "ecb2_c": np.asarray(ec_b2, np.float32).reshape(H, 1),
        "ginb1_c": np.asarray(gin_b1, np.float32).reshape(128, 1),
        "ginb2_c": np.asarray(gin_b2, np.float32).reshape(H, 1),
        "gateb1_c": np.asarray(gate_b1, np.float32).reshape(128, 1),
        "iota64": np.tile(np.arange(G, dtype=np.float32), (128, 1)),
    }
    per_core = []
    for r in range(R):
        d = dict(const)
        d.update({"xT": xs[r], "dinv": dinvs[r], "bids": bidss[r],
                  "idx1": np.ascontiguousarray(idx1[r]),
                  "idx2ec": np.ascontiguousarray(idx2ec[r]),
                  "idx2gin": np.ascontiguousarray(idx2gin[r])})
        per_core.append(d)
    meta = {
        "K1": [int(k) for k in K1], "off1": [int(o) for o in off1],
        "K2": [int(k) for k in K2], "off2": [int(o) for o in off2],
        "S1": int(off1[-1]), "S2": int(off2[-1]),
        "gate_b2": float(np.asarray(gate_b2).reshape(-1)[0]),
        "perm": perm,
    }
    return per_core, meta


def _tree_fold(nc, vfn, K, op):
    """In-place fold of K slots to slot 0: vfn(a, n) -> AP [128, n, D]."""
    k = K
    while k > 1:
        h = k // 2
        nc.vector.tensor_tensor(out=vfn(0, h), in0=vfn(0, h),
                                in1=vfn(k - h, h), op=op)
        k -= h


def _build(meta):
    K1, off1, S1 = meta["K1"], meta["off1"], meta["S1"]
    K2, off2, S2 = meta["K2"], meta["off2"], meta["S2"]
    gate_b2 = meta["gate_b2"]

    nc = bacc.Bacc("TRN2", target_bir_lowering=False, debug=False,
                   num_devices=R)

    def din(name, shape, dt=f32):
        return nc.dram_tensor(name, shape, dt, kind="ExternalInput")

    xP = din("xT", [IN, NPCP])
    dinvP = din("dinv", [128, NB])
    bidsP = din("bids", [128, NB])
    idx1P = din("idx1", [128, S1], i32)
    idx2ecP = din("idx2ec", [128, S2], i32)
    idx2ginP = din("idx2gin", [128, S2], i32)
    wP = {n: din(n, list(s)) for n, s in [
        ("gcn_W", (IN, H)), ("gat_W", (H, H)), ("B_src", (H, HEADS)),
        ("B_dst", (H, HEADS)), ("W1b", (H, H)), ("W1d", (H, H)),
        ("ec_W2", (H, H)), ("gin_W1", (H, 128)), ("gin_W2", (128, H)),
        ("gate_W1", (H, 128)), ("gate_W2", (H, 1)), ("fc_W", (H, OUT)),
        ("fc_b", (1, OUT)), ("gcnb_bc", (128, H)), ("gatb_bc", (128, H)),
        ("ecb1_bc", (128, H)), ("ecb2_c", (H, 1)), ("ginb1_c", (128, 1)),
        ("ginb2_c", (H, 1)), ("gateb1_c", (128, 1)), ("iota64", (128, G)),
    ]}
    outP = nc.dram_tensor("out", [G, OUT], f32, kind="ExternalOutput")

    # internal DRAM: per-chunk local shards + replicated Shared tables
    def shards(name, w):
        return [nc.dram_tensor(f"{name}_c{c}", [int(_chunk_rows[c]), w], bf16)
                for c in range(len(CHUNKS))]
    z_sh, g_sh, u_sh, h3_sh = (shards("z_sh", H), shards("g_sh", GW),
                               shards("u_sh", H), shards("h3_sh", H))
    z_tab = nc.dram_tensor("z_tab", [TABR, H], bf16, addr_space="Shared")
    g_tab = nc.dram_tensor("g_tab", [TABR, GW], bf16, addr_space="Shared")
    u_tab = nc.dram_tensor("u_tab", [TABR, H], bf16, addr_space="Shared")
    h3_tab = nc.dram_tensor("h3_tab", [TABR, H], bf16, addr_space="Shared")
    v_loc = nc.dram_tensor("v_loc", [NPCP, H], f32)
    h3_loc = nc.dram_tensor("h3_loc", [NPCP, H], bf16)
    ar_in = nc.dram_tensor("ar_in", [G, 132], f32)
    ar_out = nc.dram_tensor("ar_out", [G, 132], f32, addr_space="Shared")

    def shard_rc(sh_list, b):
        """(shard tensor, local row base) for block b."""
        c = int(_chunk_of_block[b])
        return sh_list[c], (b - int(_chunk_start[c])) * 128

    def ag(sh_list, tab, w, c):
        base = int(_chunk_tab_base[c])
        rows = R * int(_chunk_rows[c])
        nc.gpsimd.collective_compute(
            "AllGather", OP.bypass, ins=[sh_list[c][:, :].opt()],
            outs=[tab[base:base + rows, :].opt()], replica_groups=REPL)

    with tile.TileContext(nc) as tc:
        with tc.tile_pool(name="cst", bufs=1) as cst, \
             tc.tile_pool(name="wrk", bufs=3) as wrk, \
             tc.tile_pool(name="gth", bufs=3) as gth, \
             tc.tile_pool(name="ps128", bufs=2, space="PSUM") as ps128, \
             tc.tile_pool(name="psa", bufs=1, space="PSUM") as psa, \
             tc.tile_pool(name="ps512", bufs=3, space="PSUM") as ps512, \
             tc.tile_pool(name="psacc", bufs=1, space="PSUM") as psacc:

            ident = cst.tile([128, 128], f32)
            make_identity(nc, ident[:])
            W = {}
            for n, t in wP.items():
                wt = cst.tile(list(t.shape), f32, name=f"w_{n}")
                nc.sync.dma_start(out=wt[:], in_=t[:, :])
                W[n] = wt
            dinv_t = cst.tile([128, NB], f32)
            nc.sync.dma_start(out=dinv_t[:], in_=dinvP[:, :])
            bids_t = cst.tile([128, NB], f32)
            nc.sync.dma_start(out=bids_t[:], in_=bidsP[:, :])
            adst_all = cst.tile([128, 4 * NB], f32)
            ones_t = cst.tile([128, 1], f32)
            nc.vector.memset(ones_t[:], 1.0)
            ones_row = cst.tile([1, G], f32)
            nc.vector.memset(ones_row[:], 1.0)
            negrow = cst.tile([1, 128], bf16)
            nc.vector.memset(negrow[:], NEG)
            zrow = cst.tile([1, 128], bf16)
            nc.vector.memset(zrow[:], 0.0)
            gb2_t = cst.tile([1, 1], f32)
            nc.vector.memset(gb2_t[:], gate_b2)

            def transpose128(src_ap, name):
                pt = ps128.tile([128, 128], f32, tag="ps128", name=f"pt_{name}")
                nc.tensor.transpose(out=pt[:], in_=src_ap, identity=ident[:])
                st = wrk.tile([128, 128], f32, tag=f"tr_{name}", name=f"tr_{name}")
                nc.vector.tensor_copy(out=st[:], in_=pt[:])
                return st

            # ---------------- stage 0: z = (x @ gcn_W) * dinv[src] ---------
            for b in range(NB):
                xT = wrk.tile([128, 128], f32, name="xT")
                nc.sync.dma_start(out=xT[:],
                                  in_=xP[:, b * 128:(b + 1) * 128])
                pz = ps128.tile([128, H], f32, tag="ps128", name="pz")
                nc.tensor.matmul(out=pz[:], lhsT=xT[:], rhs=W["gcn_W"][:],
                                 start=True, stop=True)
                zr = wrk.tile([128, H], bf16, name="zr")
                nc.vector.tensor_scalar_mul(zr[:], pz[:], dinv_t[:, b:b + 1])
                sh, rb = shard_rc(z_sh, b)
                nc.sync.dma_start(out=sh[rb:rb + 128, :], in_=zr[:])
            for c in range(len(CHUNKS)):
                ag(z_sh, z_tab, H, c)

            # ------------- stage 1: GCN aggregate + GAT prep ---------------
            for b in range(NB):
                K = K1[b]
                idxt = wrk.tile([128, K], i32, tag="idxt", name="idxt")
                nc.sync.dma_start(out=idxt[:],
                                  in_=idx1P[:, off1[b]:off1[b] + K])
                zt = gth.tile([128, K * H], bf16, tag="gath", name="zt")
                nc.gpsimd.indirect_dma_start(
                    out=zt[:], out_offset=None, in_=z_tab[:, :],
                    in_offset=bass.IndirectOffsetOnAxis(ap=idxt[:, :], axis=0))
                zf = gth.tile([128, K * H], f32, tag="gfold", name="zf")
                nc.vector.tensor_copy(out=zf[:], in_=zt[:])
                z3 = zf[:].rearrange("p (k d) -> p k d", k=K)
                _tree_fold(nc, lambda a, n: z3[:, a:a + n, :], K, OP.add)
                h1 = wrk.tile([128, H], f32, name="h1")
                nc.vector.tensor_scalar_mul(h1[:], zf[:, :H],
                                            dinv_t[:, b:b + 1])
                nc.vector.tensor_tensor(out=h1[:], in0=h1[:],
                                        in1=W["gcnb_bc"][:], op=OP.add)
                nc.vector.tensor_scalar_max(h1[:], h1[:], 0.0)
                h1T = transpose128(h1[:], "h1T")
                pg = ps128.tile([128, H], f32, tag="ps128", name="pg")
                nc.tensor.matmul(out=pg[:], lhsT=h1T[:], rhs=W["gat_W"][:],
                                 start=True, stop=True)
                pa = psa.tile([128, 2 * HEADS], f32, tag="psA", name="pa")
                nc.tensor.matmul(out=pa[:, :HEADS], lhsT=h1T[:],
                                 rhs=W["B_src"][:], start=True, stop=True)
                nc.tensor.matmul(out=pa[:, HEADS:], lhsT=h1T[:],
                                 rhs=W["B_dst"][:], start=True, stop=True)
                gst = wrk.tile([128, GW], bf16, name="gst")
                nc.vector.tensor_copy(out=gst[:, :H], in_=pg[:])
                nc.vector.tensor_copy(out=gst[:, H:H + HEADS],
                                      in_=pa[:, :HEADS])
                nc.vector.memset(gst[:, H + HEADS:], 0.0)
                nc.vector.tensor_copy(out=adst_all[:, 4 * b:4 * b + 4],
                                      in_=pa[:, HEADS:])
                sh, rb = shard_rc(g_sh, b)
                nc.sync.dma_start(out=sh[rb:rb + 128, :], in_=gst[:])
            # sentinel: a_src = NEG on the last dummy row (block NB-1)
            shS, rbS = shard_rc(g_sh, NB - 1)
            nc.sync.dma_start(out=shS[rbS + 127:rbS + 128, H:H + 4],
                              in_=negrow[:1, :4])
            for c in range(len(CHUNKS)):
                ag(g_sh, g_tab, GW, c)

            # ------------- stage 2: GAT aggregate + u/v prep ---------------
            for b in range(NB):
                K = K1[b]
                idxt = wrk.tile([128, K], i32, tag="idxt", name="idxt2")
                nc.sync.dma_start(out=idxt[:],
                                  in_=idx1P[:, off1[b]:off1[b] + K])
                gt = gth.tile([128, K * GW], bf16, tag="gath", name="gt")
                nc.gpsimd.indirect_dma_start(
                    out=gt[:], out_offset=None, in_=g_tab[:, :],
                    in_offset=bass.IndirectOffsetOnAxis(ap=idxt[:, :], axis=0))
                g3 = gt[:].rearrange("p (k w) -> p k w", k=K)
                asf = wrk.tile([128, K * HEADS], f32, tag="asf", name="asf")
                a3 = asf[:].rearrange("p (k h) -> p k h", k=K)
                nc.vector.tensor_copy(out=a3, in_=g3[:, :, H:H + HEADS])
                et = wrk.tile([128, K * HEADS], f32, tag="et", name="et")
                e3 = et[:].rearrange("p (k h) -> p k h", k=K)
                nc.vector.tensor_tensor(
                    out=e3, in0=a3,
                    in1=adst_all[:, 4 * b:4 * b + 4][:, None, :]
                    .to_broadcast([128, K, HEADS]), op=OP.add)
                lt = wrk.tile([128, K * HEADS], f32, tag="lt", name="lt")
                nc.vector.tensor_scalar_mul(lt[:], et[:], 0.2)
                nc.vector.tensor_tensor(out=et[:], in0=et[:], in1=lt[:],
                                        op=OP.max)
                nc.scalar.activation(et[:], et[:], AF.Exp)
                # weight g rows by exp(e) per head, then fold sums
                wtf = gth.tile([128, K * H], f32, tag="gfold", name="wtf")
                w3 = wtf[:].rearrange("p (k d) -> p k d", k=K)
                g4 = g3[:, :, :H].rearrange("p k (h c) -> p k h c", h=HEADS)
                w4 = w3.rearrange("p k (h c) -> p k h c", h=HEADS)
                e4 = e3[:, :, :, None].to_broadcast([128, K, HEADS, C])
                nc.vector.tensor_tensor(out=w4, in0=g4, in1=e4, op=OP.mult)
                _tree_fold(nc, lambda a, n: w3[:, a:a + n, :], K, OP.add)
                _tree_fold(nc, lambda a, n: e3[:, a:a + n, :], K, OP.add)
                den = wrk.tile([128, HEADS], f32, name="den")
                nc.vector.tensor_scalar_add(den[:], et[:, :HEADS], 1e-16)
                rd = wrk.tile([128, HEADS], f32, name="rd")
                nc.vector.reciprocal(rd[:], den[:])
                h2 = wrk.tile([128, H], f32, name="h2")
                h2v = h2[:].rearrange("p (h c) -> p h c", h=HEADS)
                nc.vector.tensor_tensor(
                    out=h2v, in0=wtf[:, :H].rearrange("p (h c) -> p h c",
                                                      h=HEADS),
                    in1=rd[:][:, :, None].to_broadcast([128, HEADS, C]),
                    op=OP.mult)
                nc.vector.tensor_tensor(out=h2[:], in0=h2[:],
                                        in1=W["gatb_bc"][:], op=OP.add)
                # elu
                ng = wrk.tile([128, H], f32, name="ng")
                nc.vector.tensor_scalar_min(ng[:], h2[:], 0.0)
                nc.scalar.activation(ng[:], ng[:], AF.Exp)
                nc.vector.tensor_scalar(out=h2[:], in0=h2[:], scalar1=0.0,
                                        scalar2=-1.0, op0=OP.max, op1=OP.add)
                nc.vector.tensor_tensor(out=h2[:], in0=h2[:], in1=ng[:],
                                        op=OP.add)
                h2T = transpose128(h2[:], "h2T")
                pu = ps128.tile([128, H], f32, tag="ps128", name="pu")
                nc.tensor.matmul(out=pu[:], lhsT=h2T[:], rhs=W["W1b"][:],
                                 start=True, stop=True)
                ur = wrk.tile([128, H], bf16, name="ur")
                nc.vector.tensor_copy(out=ur[:], in_=pu[:])
                sh, rb = shard_rc(u_sh, b)
                nc.sync.dma_start(out=sh[rb:rb + 128, :], in_=ur[:])
                pv = ps128.tile([128, H], f32, tag="ps128", name="pv")
                nc.tensor.matmul(out=pv[:], lhsT=h2T[:], rhs=W["W1d"][:],
                                 start=True, stop=True)
                vr = wrk.tile([128, H], f32, name="vr")
                nc.vector.tensor_tensor(out=vr[:], in0=pv[:],
                                        in1=W["ecb1_bc"][:], op=OP.add)
                nc.sync.dma_start(out=v_loc[b * 128:(b + 1) * 128, :],
                                  in_=vr[:])
            shS, rbS = shard_rc(u_sh, NB - 1)
            nc.sync.dma_start(out=shS[rbS + 127:rbS + 128, :],
                              in_=negrow[:1, :])
            for c in range(len(CHUNKS)):
                ag(u_sh, u_tab, H, c)

            # ---------------- stage 3: EdgeConv ----------------------------
            for b in range(NB):
                K = K2[b]
                accT = wrk.tile([128, H], f32, name="accT")
                nc.vector.memset(accT[:], NEG)
                if K > 0:
                    idxt = wrk.tile([128, K], i32, tag="idxt", name="idxt3")
                    nc.sync.dma_start(out=idxt[:],
                                      in_=idx2ecP[:, off2[b]:off2[b] + K])
                    ut = gth.tile([128, K * H], bf16, tag="gath", name="ut")
                    nc.gpsimd.indirect_dma_start(
                        out=ut[:], out_offset=None, in_=u_tab[:, :],
                        in_offset=bass.IndirectOffsetOnAxis(ap=idxt[:, :],
                                                            axis=0))
                    vr = wrk.tile([128, H], f32, name="vr3")
                    nc.sync.dma_start(out=vr[:],
                                      in_=v_loc[b * 128:(b + 1) * 128, :])
                    uf = gth.tile([128, K * H], f32, tag="gfold", name="uf")
                    u3 = uf[:].rearrange("p (k d) -> p k d", k=K)
                    nc.vector.tensor_tensor(
                        out=u3, in0=ut[:].rearrange("p (k d) -> p k d", k=K),
                        in1=vr[:][:, None, :].to_broadcast([128, K, H]),
                        op=OP.add)
                    k0 = 0
                    while k0 < K:
                        q = min(4, K - k0)
                        pt = ps512.tile([128, q * 128], f32, tag="ps512",
                                        name="ec_pt")
                        for j in range(q):
                            nc.tensor.transpose(
                                out=pt[:, j * 128:(j + 1) * 128],
                                in_=u3[:, k0 + j, :], identity=ident[:])
                        m1 = wrk.tile([128, q * 128], f32, tag="m1", name="m1")
                        nc.scalar.activation(m1[:], pt[:], AF.Relu)
                        pm = ps512.tile([128, q * 128], f32, tag="ps512",
                                        name="ec_pm")
                        nc.tensor.matmul(out=pm[:], lhsT=W["ec_W2"][:],
                                         rhs=m1[:], start=True, stop=True)
                        for j in range(q):
                            nc.vector.tensor_tensor(
                                out=accT[:], in0=accT[:],
                                in1=pm[:, j * 128:(j + 1) * 128], op=OP.max)
                        k0 += q
                h3T = wrk.tile([128, H], f32, name="h3T")
                nc.scalar.activation(h3T[:], accT[:], AF.Relu,
                                     bias=W["ecb2_c"][:, :1])
                ph3 = ps128.tile([128, 128], f32, tag="ps128", name="ph3")
                nc.tensor.transpose(out=ph3[:], in_=h3T[:], identity=ident[:])
                h3r = wrk.tile([128, H], bf16, name="h3r")
                nc.vector.tensor_copy(out=h3r[:], in_=ph3[:])
                sh, rb = shard_rc(h3_sh, b)
                nc.sync.dma_start(out=sh[rb:rb + 128, :], in_=h3r[:])
                nc.sync.dma_start(out=h3_loc[b * 128:(b + 1) * 128, :],
                                  in_=h3r[:])
            shS, rbS = shard_rc(h3_sh, NB - 1)
            nc.sync.dma_start(out=shS[rbS + 127:rbS + 128, :], in_=zrow[:1, :])
            nc.sync.dma_start(out=h3_loc[NPCP - 1:NPCP, :], in_=zrow[:1, :])
            for c in range(len(CHUNKS)):
                ag(h3_sh, h3_tab, H, c)

            # ---------------- stage 4: GIN + gated pooling -----------------
            pp = psacc.tile([G, H], f32, name="pp")
            pd = psacc.tile([G, 1], f32, name="pd")
            for b in range(NB):
                K = K2[b]
                s = wrk.tile([128, H], f32, name="s")
                selfr = wrk.tile([128, H], bf16, name="selfr")
                nc.sync.dma_start(out=selfr[:],
                                  in_=h3_loc[b * 128:(b + 1) * 128, :])
                if K > 0:
                    idxt = wrk.tile([128, K], i32, tag="idxt", name="idxt4")
                    nc.sync.dma_start(out=idxt[:],
                                      in_=idx2ginP[:, off2[b]:off2[b] + K])
                    st = gth.tile([128, K * H], bf16, tag="gath", name="st")
                    nc.gpsimd.indirect_dma_start(
                        out=st[:], out_offset=None, in_=h3_tab[:, :],
                        in_offset=bass.IndirectOffsetOnAxis(ap=idxt[:, :],
                                                            axis=0))
                    sf = gth.tile([128, K * H], f32, tag="gfold", name="sf")
                    nc.vector.tensor_copy(out=sf[:], in_=st[:])
                    s3 = sf[:].rearrange("p (k d) -> p k d", k=K)
                    _tree_fold(nc, lambda a, n: s3[:, a:a + n, :], K, OP.add)
                    nc.vector.tensor_tensor(out=s[:], in0=sf[:, :H],
                                            in1=selfr[:], op=OP.add)
                else:
                    nc.vector.tensor_copy(out=s[:], in_=selfr[:])
                sT = transpose128(s[:], "sT")
                p1 = ps128.tile([128, 128], f32, tag="ps128", name="p1")
                nc.tensor.matmul(out=p1[:], lhsT=W["gin_W1"][:], rhs=sT[:],
                                 start=True, stop=True)
                t1 = wrk.tile([128, 128], f32, name="t1")
                nc.scalar.activation(t1[:], p1[:], AF.Relu,
                                     bias=W["ginb1_c"][:, :1])
                p2 = ps128.tile([128, H], f32, tag="ps128", name="p2")
                nc.tensor.matmul(out=p2[:], lhsT=W["gin_W2"][:], rhs=t1[:],
                                 start=True, stop=True)
                h4T = wrk.tile([128, H], f32, name="h4T")
                nc.scalar.activation(h4T[:], p2[:], AF.Relu,
                                     bias=W["ginb2_c"][:, :1])
                pg1 = ps128.tile([128, 128], f32, tag="ps128", name="pg1")
                nc.tensor.matmul(out=pg1[:], lhsT=W["gate_W1"][:], rhs=h4T[:],
                                 start=True, stop=True)
                g1 = wrk.tile([128, 128], f32, name="g1")
                nc.scalar.activation(g1[:], pg1[:], AF.Relu,
                                     bias=W["gateb1_c"][:, :1])
                pg2 = psa.tile([1, 128], f32, tag="psA", name="pg2")
                nc.tensor.matmul(out=pg2[:], lhsT=W["gate_W2"][:], rhs=g1[:],
                                 start=True, stop=True)
                egT = wrk.tile([1, 128], f32, name="egT")
                nc.scalar.activation(egT[:], pg2[:], AF.Exp,
                                     bias=gb2_t[:1, :1])
                h4r = transpose128(h4T[:], "h4r")
                pe = psa.tile([128, 1], f32, tag="psA", name="pe")
                nc.tensor.transpose(out=pe[:], in_=egT[:1, :],
                                    identity=ident[:1, :1])
                egc = wrk.tile([128, 1], f32, name="egc")
                nc.vector.tensor_copy(out=egc[:], in_=pe[:])
                oh = wrk.tile([128, G], f32, name="oh")
                nc.vector.tensor_tensor(
                    out=oh[:], in0=bids_t[:, b:b + 1].to_broadcast([128, G]),
                    in1=W["iota64"][:], op=OP.is_equal)
                nc.vector.tensor_scalar_mul(oh[:], oh[:], egc[:, :1])
                nc.tensor.matmul(out=pp[:], lhsT=oh[:], rhs=h4r[:],
                                 start=(b == 0), stop=(b == NB - 1))
                nc.tensor.matmul(out=pd[:], lhsT=oh[:], rhs=ones_t[:, :1],
                                 start=(b == 0), stop=(b == NB - 1))

            # pooled partials -> AllReduce -> final dense head (all cores)
            pl = wrk.tile([G, 132], f32, name="pl")
            nc.vector.memset(pl[:], 0.0)
            nc.vector.tensor_copy(out=pl[:, :H], in_=pp[:])
            nc.vector.tensor_copy(out=pl[:, H:H + 1], in_=pd[:])
            nc.sync.dma_start(out=ar_in[:, :], in_=pl[:])
            nc.gpsimd.collective_compute(
                "AllReduce", OP.add, ins=[ar_in[:, :].opt()],
                outs=[ar_out[:, :].opt()], replica_groups=REPL)
            ar = wrk.tile([G, 132], f32, name="ar")
            nc.sync.dma_start(out=ar[:], in_=ar_out[:, :])
            rdn = wrk.tile([G, 1], f32, name="rdn")
            nc.vector.reciprocal(rdn[:], ar[:, H:H + 1])
            pooled = wrk.tile([G, H], f32, name="pooled")
            nc.vector.tensor_scalar_mul(pooled[:], ar[:, :H], rdn[:, :1])
            ppT = ps128.tile([128, G], f32, tag="ps128", name="ppT")
            nc.tensor.transpose(out=ppT[:], in_=pooled[:G, :],
                                identity=ident[:G, :G])
            plT = wrk.tile([128, G], f32, name="plT")
            nc.vector.tensor_copy(out=plT[:], in_=ppT[:])
            psl = psa.tile([G, OUT], f32, tag="psA", name="psl")
            nc.tensor.matmul(out=psl[:], lhsT=plT[:], rhs=W["fc_W"][:],
                             start=True, stop=False)
            nc.tensor.matmul(out=psl[:], lhsT=ones_row[:1, :],
                             rhs=W["fc_b"][:1, :], start=False, stop=True)
            rmx = wrk.tile([G, 1], f32, name="rmx")
            nc.vector.tensor_reduce(out=rmx[:], in_=psl[:],
                                    axis=mybir.AxisListType.X, op=OP.max)
            xs = wrk.tile([G, OUT], f32, name="xs")
            nc.vector.tensor_scalar(out=xs[:], in0=psl[:], scalar1=rmx[:, :1],
                                    scalar2=None, op0=OP.subtract)
            ex = wrk.tile([G, OUT], f32, name="ex")
            ssum = wrk.tile([G, 1], f32, name="ssum")
            nc.scalar.activation(ex[:], xs[:], AF.Exp, accum_out=ssum[:, :1])
            lg = wrk.tile([G, 1], f32, name="lg")
            nc.scalar.activation(lg[:], ssum[:], AF.Ln)
            fin = wrk.tile([G, OUT], f32, name="fin")
            nc.vector.tensor_scalar(out=fin[:], in0=xs[:], scalar1=lg[:, :1],
                                    scalar2=None, op0=OP.subtract)
            nc.sync.dma_start(out=outP[:, :], in_=fin[:])

    nc.compile()
    return nc


_CACHE = {}


def kernel(**inputs) -> np.ndarray:
    per_core, meta = _preprocess(**inputs)
    key = (tuple(meta["K1"]), tuple(meta["K2"]))
    if key not in _CACHE:
        _CACHE[key] = _build(meta)
    nc = _CACHE[key]
    res = run_bass_kernel_spmd(nc, per_core, list(range(R)))
    return np.asarray(res.results[0]["out"], np.float32)


if __name__ == "__main__":
    import reference
    inputs = {k: np.asarray(v) for k, v in reference.setup_inputs().items()}
    got = kernel(**inputs)
    print(got[:4])

